# revision 18
# baseline (speedup 1.0000x reference)
"""Trainium2 Bass kernel for nn_CrossAttention (B=2, N=M=2048, DIM=512, H=8, DH=64).

Sharding: token-parallel across 8 cores. Core c handles batch b = c // 4 and
query rows [ (c%4)*512, (c%4+1)*512 ) of that batch. Each core recomputes K/V
for its batch from the full context (no cross-core communication).

Per-core pipeline (all on one NeuronCore, fp32 data, float32r matmuls):
  1. LayerNorm(x_slice)                       [q, D] layout
  2. PE-transpose xn and context              -> [D, q] / [D, keys]
  3. qT = Wq.T @ xnT (scaled by 1/64)         [inner, q]
     kT = Wk.T @ ctxT                         [inner, keys]
     v  = ctxT.T @ Wv, masked rows, + mask column -> v_aug [keys, 65] per head
  4. per head: simT = kT_h.T @ qT_h           [keys, q]   (PSUM)
     expT = exp(simT)                         (ACT, no max-subtraction: logits
                                               are O(0.1) by construction)
     outT += v_aug.T @ expT                   [65, q]: rows 0-63 = sum exp*v,
                                               row 64 = sum exp*mask (denom)
     normalize: outTn = outT[0:64] * bcast(1/outT[64])
  5. final = outTn.T @ Wo, LayerNorm, store   [q, D]
Masking is folded into V: masked keys contribute exp*0 to both numerator and
denominator, which is exactly softmax(where(mask, sim, -inf)) @ v.
"""

import numpy as np

import concourse.bass as bass
import concourse.tile as tile
from concourse import bacc, mybir
from concourse.bass_utils import run_bass_kernel_spmd
from concourse.masks import make_identity

F32 = mybir.dt.float32
F32R = mybir.dt.float32r
BF16 = mybir.dt.bfloat16
AOP = mybir.AluOpType
AFT = mybir.ActivationFunctionType

B, N, M, DIM, H, DH = 2, 2048, 2048, 512, 8, 64
INNER = H * DH
T = 512           # query tokens per core
NCORES = 8
SCALE2 = 1.0 / DH  # q*DH^-0.5, k*DH^-0.5 folded into one factor on q
EPS = 1e-5

P = 128
TT_ = T // P       # 4 query tiles
DC = DIM // P      # 4 contraction chunks
IC = INNER // P    # 4 inner chunks
KT = M // P        # 16 key tiles
JG = 2             # key tiles per exp group


def build_program():
    nc = bacc.Bacc("TRN2", target_bir_lowering=False, debug=False,
                   num_devices=NCORES)

    x_d = nc.dram_tensor("x_s", [T, DIM], F32, kind="ExternalInput")
    ctx_d = nc.dram_tensor("ctx", [M, DIM], F32, kind="ExternalInput")
    mask_d = nc.dram_tensor("maskf", [M], F32, kind="ExternalInput")
    wq_d = nc.dram_tensor("Wq", [DIM, INNER], F32, kind="ExternalInput")
    wk_d = nc.dram_tensor("Wk", [DIM, INNER], F32, kind="ExternalInput")
    wv_d = nc.dram_tensor("Wv", [DIM, INNER], F32, kind="ExternalInput")
    wo_d = nc.dram_tensor("Wo", [INNER, DIM], F32, kind="ExternalInput")
    lng_d = nc.dram_tensor("ln_g", [DIM], F32, kind="ExternalInput")
    lnb_d = nc.dram_tensor("ln_b", [DIM], F32, kind="ExternalInput")
    log_d = nc.dram_tensor("lno_g", [DIM], F32, kind="ExternalInput")
    lob_d = nc.dram_tensor("lno_b", [DIM], F32, kind="ExternalInput")
    y_d = nc.dram_tensor("y", [T, DIM], F32, kind="ExternalOutput")

    def pbcast(vec_dram):
        ap = vec_dram.ap()
        return bass.AP(tensor=ap.tensor, offset=ap.offset, ap=[[0, P], ap.ap[0]])

    def fbcast(col_ap, n):
        # [P, 1] -> [P, n, 1] with stride-0 middle dim
        return bass.AP(tensor=col_ap.tensor, offset=col_ap.offset,
                       ap=[col_ap.ap[0], [0, n], col_ap.ap[1]])

    with tile.TileContext(nc) as tc:
        with (
            tc.tile_pool(name="const", bufs=1) as cpool,
            tc.tile_pool(name="data", bufs=1) as dpool,
            tc.tile_pool(name="ctxs", bufs=3) as ctxpool,
            tc.tile_pool(name="expp", bufs=2) as epool,
            tc.tile_pool(name="wst", bufs=1) as wstpool,
            tc.tile_pool(name="yp", bufs=2) as ypool,
            tc.tile_pool(name="bcp", bufs=1) as bcpool,
            tc.tile_pool(name="chp", bufs=2) as chpool,
            tc.tile_pool(name="small", bufs=6) as spool,
            tc.tile_pool(name="ps", bufs=4, space="PSUM") as ps,
            tc.tile_pool(name="ps2", bufs=2, space="PSUM") as ps2,
        ):
            # ---- constants / weights ----
            ident = cpool.tile([P, P], F32)
            make_identity(nc, ident)
            eps_t = cpool.tile([P, 1], F32)
            nc.vector.memset(eps_t, EPS)

            gb = cpool.tile([P, DIM], F32, tag="gb")
            bb = cpool.tile([P, DIM], F32, tag="bb")
            logb = cpool.tile([P, DIM], F32, tag="logb")
            lobb = cpool.tile([P, DIM], F32, tag="lobb")
            nc.sync.dma_start(out=gb, in_=pbcast(lng_d))
            nc.sync.dma_start(out=bb, in_=pbcast(lnb_d))

            mask_sb = cpool.tile([P, KT], F32, tag="mask")
            nc.gpsimd.dma_start(out=mask_sb, in_=mask_d.ap().rearrange("(kt p) -> p kt", p=P))

            wq_sb = cpool.tile([P, DC, INNER], F32R, tag="wq")
            wk_sb = cpool.tile([P, DC, INNER], F32R, tag="wk")
            wv_sb = cpool.tile([P, DC, INNER], F32R, tag="wv")
            wo_sb = cpool.tile([P, IC, DIM], F32R, tag="wo")

            def load_weights():
                # on the gpsimd (SWDGE) queue, parallel to ctx loads on sync
                for w_sb, w_d, pat in (
                    (wq_sb, wq_d, "(dc p) i -> p dc i"),
                    (wk_sb, wk_d, "(dc p) i -> p dc i"),
                    (wv_sb, wv_d, "(dc p) i -> p dc i"),
                    (wo_sb, wo_d, "(ic p) d -> p ic d"),
                ):
                    wst = wstpool.tile([P, DC, INNER], F32, tag="wstage")
                    nc.gpsimd.dma_start(out=wst, in_=w_d.ap().rearrange(pat, p=P))
                    nc.vector.tensor_copy(w_sb[:, :, :], wst)

            # ---- persistent data tiles ----
            x_sb = dpool.tile([P, TT_, DIM], F32, tag="x")
            xnT = dpool.tile([P, DC, T], F32R, tag="xnT")
            qT = dpool.tile([P, IC, T], F32R, tag="qT")
            kTt = dpool.tile([P, IC, M], F32R, tag="kT")
            ctxT = dpool.tile([P, DC, M], F32R, tag="ctxT")
            vaug = dpool.tile([P, KT, H, DH + 1], BF16, tag="vaug")
            outTn = dpool.tile([P, IC, T], F32R, tag="outTn")

            nc.sync.dma_start(out=x_sb, in_=x_d.ap().rearrange("(tt p) d -> p tt d", p=P))

            import contextlib
            stack = contextlib.ExitStack()

            def scope(name):
                stack.close()
                stack.enter_context(nc.named_scope(name))

            # ---- stage 1: LayerNorm(x) in place ----
            scope("ln1")
            for tt in range(TT_):
                xt = x_sb[:, tt, :]
                st = spool.tile([P, 6], F32, tag="st")
                mv = spool.tile([P, 2], F32, tag="mv")
                nc.vector.bn_stats(st, xt)
                nc.vector.bn_aggr(mv, st)
                std = spool.tile([P, 1], F32, tag="std")
                nc.scalar.activation(std, mv[:, 1:2], AFT.Sqrt, bias=eps_t[:, 0:1])
                rstd = spool.tile([P, 1], F32, tag="rstd")
                nc.vector.reciprocal(rstd, std)
                nc.vector.tensor_scalar(out=xt, in0=xt, scalar1=mv[:, 0:1],
                                        scalar2=rstd, op0=AOP.subtract, op1=AOP.mult)
                nc.vector.tensor_tensor(out=xt, in0=xt, in1=gb, op=AOP.mult)
                nc.vector.tensor_tensor(out=xt, in0=xt, in1=bb, op=AOP.add)

            # ---- stage 2a: transpose xn -> xnT (scaled by 1/64) ----
            scope("tpose_xn")
            for dc in range(DC):
                pt = ps.tile([P, TT_, P], F32, tag="mm")
                for tt in range(TT_):
                    nc.tensor.transpose(pt[:, tt, :], x_sb[:, tt, bass.ts(dc, P)], ident)
                nc.vector.tensor_scalar_mul(xnT[:, dc, :], pt, SCALE2)

            # ---- stage 2b: transpose context -> ctxT ----
            scope("tpose_ctx")
            for kt in range(KT):
                ct = ctxpool.tile([P, DIM], F32, tag="ctx")
                nc.sync.dma_start(out=ct, in_=ctx_d[bass.ts(kt, P), :])
                pt = ps.tile([P, DC, P], F32, tag="mm")
                for dc in range(DC):
                    nc.tensor.transpose(pt[:, dc, :], ct[:, bass.ts(dc, P)], ident)
                nc.vector.tensor_copy(ctxT[:, :, bass.ts(kt, P)], pt)

            load_weights()
            nc.gpsimd.dma_start(out=logb, in_=pbcast(log_d))
            nc.gpsimd.dma_start(out=lobb, in_=pbcast(lob_d))

            # ---- stage 3a: qT = Wq.T @ xnT ----
            scope("qproj")
            for ic in range(IC):
                pq = ps.tile([P, T], F32, tag="mm")
                for dc in range(DC):
                    nc.tensor.matmul(pq, wq_sb[:, dc, bass.ts(ic, P)],
                                     xnT[:, dc, :],
                                     start=(dc == 0), stop=(dc == DC - 1))
                nc.vector.tensor_copy(qT[:, ic, :], pq)

            # ---- stage 3b: kT = Wk.T @ ctxT ----
            scope("kproj")
            for ic in range(IC):
                for kc in range(M // T):
                    pk = ps.tile([P, T], F32, tag="mm")
                    for dc in range(DC):
                        nc.tensor.matmul(pk, wk_sb[:, dc, bass.ts(ic, P)],
                                         ctxT[:, dc, bass.ts(kc, T)],
                                         start=(dc == 0), stop=(dc == DC - 1))
                    nc.vector.tensor_copy(kTt[:, ic, bass.ts(kc, T)], pk)

            # ---- stage 3c: v = ctxT.T @ Wv, mask rows, mask column ----
            scope("vproj")
            for kt in range(KT):
                pv = ps.tile([P, INNER], F32, tag="mm")
                for dc in range(DC):
                    nc.tensor.matmul(pv, ctxT[:, dc, bass.ts(kt, P)],
                                     wv_sb[:, dc, :],
                                     start=(dc == 0), stop=(dc == DC - 1))
                nc.vector.tensor_scalar_mul(
                    vaug[:, kt, :, 0:DH],
                    pv.rearrange("p (h d) -> p h d", h=H),
                    mask_sb[:, kt:kt + 1])
                nc.vector.tensor_copy(vaug[:, kt, :, DH:DH + 1],
                                      fbcast(mask_sb[:, kt:kt + 1], H))

            # ---- stage 4: attention per head ----
            scope("attn")
            HB1 = 7  # heads in first normalization batch
            outU = dpool.tile([P, IC, T], F32, tag="xnT")  # reuses xnT's slot
            den0 = bcpool.tile([HB1, T], F32, tag="den0")
            den1 = bcpool.tile([1, T], F32, tag="den1")
            dens = [den0, den1]

            def normalize_batch(b):
                # batched exact reciprocal (rows at partitions 0..n-1)
                n = HB1 if b == 0 else 1
                recb = bcpool.tile([n, T], F32, tag=f"rec{b}")
                nc.vector.reciprocal(recb[0:n, :], dens[b][0:n, :])
                for h in range(0 if b == 0 else HB1, HB1 if b == 0 else H):
                    ic, off = h // 2, (h % 2) * DH
                    r = h if b == 0 else 0
                    # DMA (no partition-start limits) moves row r to partition 0
                    rtmp = chpool.tile([1, T], F32, tag="rtmp")
                    nc.sync.dma_start(out=rtmp[0:1, :], in_=recb[r:r + 1, :])
                    bc = chpool.tile([P, T], F32, tag="bcs")
                    nc.gpsimd.partition_broadcast(bc[0:P, :], rtmp[0:1, :])
                    nc.vector.tensor_tensor(out=outTn[off:off + DH, ic, :],
                                            in0=outU[off:off + DH, ic, :],
                                            in1=bc[off:off + DH, :], op=AOP.mult)

            for h in range(H):
                ic, off = h // 2, (h % 2) * DH
                po = ps.tile([DH + 1, T], F32, tag="mm")
                for grp in range(KT // JG):
                    psim = ps2.tile([P, JG, T], F32, tag="sim")
                    for j2 in range(JG):
                        jt = grp * JG + j2
                        nc.tensor.matmul(psim[:, j2, :],
                                         kTt[off:off + DH, ic, bass.ts(jt, P)],
                                         qT[off:off + DH, ic, :],
                                         start=True, stop=True)
                    et = epool.tile([P, JG, T], BF16, tag="expT")
                    nc.scalar.activation(et, psim, AFT.Exp)
                    for j2 in range(JG):
                        jt = grp * JG + j2
                        nc.tensor.matmul(po[0:DH + 1, :],
                                         vaug[:, jt, h, :],
                                         et[:, j2, :],
                                         start=(jt == 0), stop=(jt == KT - 1))
                nc.vector.tensor_copy(outU[off:off + DH, ic, :], po[0:DH, :])
                dtmp = chpool.tile([1, T], F32, tag="dtmp")
                nc.vector.tensor_copy(dtmp[0:1, :], po[DH:DH + 1, :])
                b = 0 if h < HB1 else 1
                nc.sync.dma_start(out=dens[b][(h if b == 0 else h - HB1):(h if b == 0 else h - HB1) + 1, :],
                                  in_=dtmp[0:1, :])
                if h == HB1 - 1 or h == H - 1:
                    normalize_batch(0 if h == HB1 - 1 else 1)

            # ---- stage 5: final projection + LayerNorm ----
            scope("final")
            for qc in range(TT_):
                pf = ps.tile([P, DIM], F32, tag="mm")
                for ic in range(IC):
                    nc.tensor.matmul(pf, outTn[:, ic, bass.ts(qc, P)],
                                     wo_sb[:, ic, :],
                                     start=(ic == 0), stop=(ic == IC - 1))
                st = spool.tile([P, 6], F32, tag="st")
                mv = spool.tile([P, 2], F32, tag="mv")
                nc.vector.bn_stats(st, pf)
                nc.vector.bn_aggr(mv, st)
                std = spool.tile([P, 1], F32, tag="std")
                nc.scalar.activation(std, mv[:, 1:2], AFT.Sqrt, bias=eps_t[:, 0:1])
                rstd = spool.tile([P, 1], F32, tag="rstd")
                nc.vector.reciprocal(rstd, std)
                yt = ypool.tile([P, DIM], F32, tag="y")
                nc.vector.tensor_scalar(out=yt, in0=pf, scalar1=mv[:, 0:1],
                                        scalar2=rstd, op0=AOP.subtract, op1=AOP.mult)
                nc.vector.tensor_tensor(out=yt, in0=yt, in1=logb, op=AOP.mult)
                nc.vector.tensor_tensor(out=yt, in0=yt, in1=lobb, op=AOP.add)
                nc.sync.dma_start(out=y_d[bass.ts(qc, P), :], in_=yt)
            stack.close()

    nc.compile()
    return nc


def make_in_maps(x, context, mask, ln_g, ln_b, Wq, Wkv, Wo, lno_g, lno_b):
    x = np.asarray(x, np.float32)
    context = np.asarray(context, np.float32)
    maskf = np.asarray(mask).astype(np.float32)
    Wq = np.ascontiguousarray(np.asarray(Wq, np.float32))
    Wkv = np.asarray(Wkv, np.float32)
    Wk = np.ascontiguousarray(Wkv[:, :INNER])
    Wv = np.ascontiguousarray(Wkv[:, INNER:])
    Wo = np.ascontiguousarray(np.asarray(Wo, np.float32))
    ln_g = np.asarray(ln_g, np.float32)
    ln_b = np.asarray(ln_b, np.float32)
    lno_g = np.asarray(lno_g, np.float32)
    lno_b = np.asarray(lno_b, np.float32)

    in_maps = []
    for c in range(NCORES):
        b, q0 = c // (NCORES // B), (c % (NCORES // B)) * T
        in_maps.append({
            "x_s": np.ascontiguousarray(x[b, q0:q0 + T]),
            "ctx": np.ascontiguousarray(context[b]),
            "maskf": np.ascontiguousarray(maskf[b]),
            "Wq": Wq, "Wk": Wk, "Wv": Wv, "Wo": Wo,
            "ln_g": ln_g, "ln_b": ln_b, "lno_g": lno_g, "lno_b": lno_b,
        })
    return in_maps


_NC = None


def _get_nc():
    global _NC
    if _NC is None:
        _NC = build_program()
    return _NC


def kernel(x, context, mask, ln_g, ln_b, Wq, Wkv, Wo, lno_g, lno_b, **run_kwargs):
    nc = _get_nc()
    in_maps = make_in_maps(x, context, mask, ln_g, ln_b, Wq, Wkv, Wo, lno_g, lno_b)
    res = run_bass_kernel_spmd(nc, in_maps, core_ids=list(range(NCORES)), **run_kwargs)
    out = np.empty((B, N, DIM), np.float32)
    for c in range(NCORES):
        b, q0 = c // (NCORES // B), (c % (NCORES // B)) * T
        out[b, q0:q0 + T] = res.results[c]["y"]
    if run_kwargs:
        kernel.last_results = res
    return out


# revision 20
# speedup vs baseline: 1.3255x; 1.3255x over previous
"""Trainium2 Bass kernel for nn_CrossAttention (B=2, N=M=2048, DIM=512, H=8, DH=64).

Sharding: token-parallel across 8 cores. Core c handles batch b = c // 4 and
query rows [ (c%4)*512, (c%4+1)*512 ) of that batch. Each core recomputes K/V
for its batch from the full context (no cross-core communication).

Per-core pipeline (all on one NeuronCore, fp32 data, float32r matmuls):
  1. LayerNorm(x_slice)                       [q, D] layout
  2. PE-transpose xn and context              -> [D, q] / [D, keys]
  3. qT = Wq.T @ xnT (scaled by 1/64)         [inner, q]
     kT = Wk.T @ ctxT                         [inner, keys]
     v  = ctxT.T @ Wv, masked rows, + mask column -> v_aug [keys, 65] per head
  4. per head: simT = kT_h.T @ qT_h           [keys, q]   (PSUM)
     expT = exp(simT)                         (ACT, no max-subtraction: logits
                                               are O(0.1) by construction)
     outT += v_aug.T @ expT                   [65, q]: rows 0-63 = sum exp*v,
                                               row 64 = sum exp*mask (denom)
     normalize: outTn = outT[0:64] * bcast(1/outT[64])
  5. final = outTn.T @ Wo, LayerNorm, store   [q, D]
Masking is folded into V: masked keys contribute exp*0 to both numerator and
denominator, which is exactly softmax(where(mask, sim, -inf)) @ v.
"""

import numpy as np

import concourse.bass as bass
import concourse.tile as tile
from concourse import bacc, mybir
from concourse.bass_utils import run_bass_kernel_spmd
from concourse.masks import make_identity

F32 = mybir.dt.float32
F32R = mybir.dt.float32r
BF16 = mybir.dt.bfloat16
AOP = mybir.AluOpType
AFT = mybir.ActivationFunctionType

B, N, M, DIM, H, DH = 2, 2048, 2048, 512, 8, 64
INNER = H * DH
T = 512           # query tokens per core
NCORES = 8
SCALE2 = 1.0 / DH  # q*DH^-0.5, k*DH^-0.5 folded into one factor on q
EPS = 1e-5

P = 128
TT_ = T // P       # 4 query tiles
DC = DIM // P      # 4 contraction chunks
IC = INNER // P    # 4 inner chunks
KT = M // P        # 16 key tiles
JG = 2             # key tiles per exp group


def build_program():
    nc = bacc.Bacc("TRN2", target_bir_lowering=False, debug=False,
                   num_devices=NCORES)

    x_d = nc.dram_tensor("x_s", [T, DIM], F32, kind="ExternalInput")
    ctx_d = nc.dram_tensor("ctx", [M, DIM], F32, kind="ExternalInput")
    mask_d = nc.dram_tensor("maskf", [M], F32, kind="ExternalInput")
    wq_d = nc.dram_tensor("Wq", [DIM, INNER], F32, kind="ExternalInput")
    wk_d = nc.dram_tensor("Wk", [DIM, INNER], F32, kind="ExternalInput")
    wv_d = nc.dram_tensor("Wv", [DIM, INNER], F32, kind="ExternalInput")
    wo_d = nc.dram_tensor("Wo", [INNER, DIM], F32, kind="ExternalInput")
    lng_d = nc.dram_tensor("ln_g", [DIM], F32, kind="ExternalInput")
    lnb_d = nc.dram_tensor("ln_b", [DIM], F32, kind="ExternalInput")
    log_d = nc.dram_tensor("lno_g", [DIM], F32, kind="ExternalInput")
    lob_d = nc.dram_tensor("lno_b", [DIM], F32, kind="ExternalInput")
    y_d = nc.dram_tensor("y", [T, DIM], F32, kind="ExternalOutput")

    def pbcast(vec_dram):
        ap = vec_dram.ap()
        return bass.AP(tensor=ap.tensor, offset=ap.offset, ap=[[0, P], ap.ap[0]])

    def fbcast(col_ap, n):
        # [P, 1] -> [P, n, 1] with stride-0 middle dim
        return bass.AP(tensor=col_ap.tensor, offset=col_ap.offset,
                       ap=[col_ap.ap[0], [0, n], col_ap.ap[1]])

    with tile.TileContext(nc) as tc:
        with (
            tc.tile_pool(name="const", bufs=1) as cpool,
            tc.tile_pool(name="data", bufs=1) as dpool,
            tc.tile_pool(name="ctxs", bufs=4) as ctxpool,
            tc.tile_pool(name="expp", bufs=3) as epool,
            tc.tile_pool(name="wst", bufs=2) as wstpool,
            tc.tile_pool(name="yp", bufs=2) as ypool,
            tc.tile_pool(name="bcp", bufs=1) as bcpool,
            tc.tile_pool(name="chp", bufs=2) as chpool,
            tc.tile_pool(name="small", bufs=6) as spool,
            tc.tile_pool(name="ps", bufs=4, space="PSUM") as ps,
            tc.tile_pool(name="ps2", bufs=2, space="PSUM") as ps2,
        ):
            # ---- constants / weights ----
            ident = cpool.tile([P, P], F32)
            make_identity(nc, ident)
            eps_t = cpool.tile([P, 1], F32)
            nc.vector.memset(eps_t, EPS)

            gb = cpool.tile([P, DIM], F32, tag="gb")
            bb = cpool.tile([P, DIM], F32, tag="bb")
            logb = cpool.tile([P, DIM], F32, tag="logb")
            lobb = cpool.tile([P, DIM], F32, tag="lobb")
            nc.sync.dma_start(out=gb, in_=pbcast(lng_d))
            nc.sync.dma_start(out=bb, in_=pbcast(lnb_d))

            mask_sb = cpool.tile([P, KT], F32, tag="mask")
            nc.sync.dma_start(out=mask_sb, in_=mask_d.ap().rearrange("(kt p) -> p kt", p=P))

            wq_sb = cpool.tile([P, DC, INNER], F32R, tag="wq")
            wk_sb = cpool.tile([P, DC, INNER], F32R, tag="wk")
            wv_sb = cpool.tile([P, DC, INNER], F32R, tag="wv")
            wo_sb = cpool.tile([P, IC, DIM], F32R, tag="wo")

            def load_weights(pairs):
                for w_sb, w_d, pat in pairs:
                    wst = wstpool.tile([P, DC, INNER], F32, tag="wstage")
                    nc.sync.dma_start(out=wst, in_=w_d.ap().rearrange(pat, p=P))
                    nc.vector.tensor_copy(w_sb[:, :, :], wst)

            # ---- persistent data tiles ----
            x_sb = dpool.tile([P, TT_, DIM], F32, tag="x")
            xnT = dpool.tile([P, DC, T], F32R, tag="xnT")
            qT = dpool.tile([P, IC, T], F32R, tag="qT")
            kTt = dpool.tile([P, IC, M], F32R, tag="kT")
            ctxT = dpool.tile([P, DC, M], F32R, tag="ctxT")
            vaug = dpool.tile([P, KT, H, DH + 1], BF16, tag="vaug")
            outTn = dpool.tile([P, IC, T], F32R, tag="outTn")

            nc.sync.dma_start(out=x_sb, in_=x_d.ap().rearrange("(tt p) d -> p tt d", p=P))

            import contextlib
            stack = contextlib.ExitStack()

            def scope(name):
                stack.close()
                stack.enter_context(nc.named_scope(name))

            # ---- stage 1: LayerNorm(x) in place ----
            scope("ln1")
            for tt in range(TT_):
                xt = x_sb[:, tt, :]
                st = spool.tile([P, 6], F32, tag="st")
                mv = spool.tile([P, 2], F32, tag="mv")
                nc.vector.bn_stats(st, xt)
                nc.vector.bn_aggr(mv, st)
                std = spool.tile([P, 1], F32, tag="std")
                nc.scalar.activation(std, mv[:, 1:2], AFT.Sqrt, bias=eps_t[:, 0:1])
                rstd = spool.tile([P, 1], F32, tag="rstd")
                nc.vector.reciprocal(rstd, std)
                nc.vector.tensor_scalar(out=xt, in0=xt, scalar1=mv[:, 0:1],
                                        scalar2=rstd, op0=AOP.subtract, op1=AOP.mult)
                nc.vector.tensor_tensor(out=xt, in0=xt, in1=gb, op=AOP.mult)
                nc.vector.tensor_tensor(out=xt, in0=xt, in1=bb, op=AOP.add)

            # ---- stage 2a: transpose xn -> xnT (scaled by 1/64) ----
            scope("tpose_xn")
            for dc in range(DC):
                pt = ps.tile([P, TT_, P], F32, tag="mm")
                for tt in range(TT_):
                    nc.tensor.transpose(pt[:, tt, :], x_sb[:, tt, bass.ts(dc, P)], ident)
                nc.vector.tensor_scalar_mul(xnT[:, dc, :], pt, SCALE2)

            # ---- stage 2b: transpose context -> ctxT ----
            scope("tpose_ctx")
            for kt in range(KT):
                ct = ctxpool.tile([P, DIM], F32, tag="ctx")
                nc.sync.dma_start(out=ct, in_=ctx_d[bass.ts(kt, P), :])
                pt = ps.tile([P, DC, P], F32, tag="mm")
                for dc in range(DC):
                    nc.tensor.transpose(pt[:, dc, :], ct[:, bass.ts(dc, P)], ident)
                nc.vector.tensor_copy(ctxT[:, :, bass.ts(kt, P)], pt)
                if kt == KT - 2:
                    load_weights([(wq_sb, wq_d, "(dc p) i -> p dc i"),
                                  (wk_sb, wk_d, "(dc p) i -> p dc i")])

            load_weights([(wv_sb, wv_d, "(dc p) i -> p dc i"),
                          (wo_sb, wo_d, "(ic p) d -> p ic d")])
            nc.sync.dma_start(out=logb, in_=pbcast(log_d))
            nc.sync.dma_start(out=lobb, in_=pbcast(lob_d))

            # ---- stage 3a: qT = Wq.T @ xnT ----
            scope("qproj")
            for ic in range(IC):
                pq = ps.tile([P, T], F32, tag="mm")
                for dc in range(DC):
                    nc.tensor.matmul(pq, wq_sb[:, dc, bass.ts(ic, P)],
                                     xnT[:, dc, :],
                                     start=(dc == 0), stop=(dc == DC - 1))
                nc.vector.tensor_copy(qT[:, ic, :], pq)

            # ---- stage 3b: kT = Wk.T @ ctxT ----
            scope("kproj")
            for ic in range(IC):
                for kc in range(M // T):
                    pk = ps.tile([P, T], F32, tag="mm")
                    for dc in range(DC):
                        nc.tensor.matmul(pk, wk_sb[:, dc, bass.ts(ic, P)],
                                         ctxT[:, dc, bass.ts(kc, T)],
                                         start=(dc == 0), stop=(dc == DC - 1))
                    nc.vector.tensor_copy(kTt[:, ic, bass.ts(kc, T)], pk)

            # ---- stage 3c: v = ctxT.T @ Wv, mask rows, mask column ----
            scope("vproj")
            for kt in range(KT):
                pv = ps.tile([P, INNER], F32, tag="mm")
                for dc in range(DC):
                    nc.tensor.matmul(pv, ctxT[:, dc, bass.ts(kt, P)],
                                     wv_sb[:, dc, :],
                                     start=(dc == 0), stop=(dc == DC - 1))
                nc.vector.tensor_scalar_mul(
                    vaug[:, kt, :, 0:DH],
                    pv.rearrange("p (h d) -> p h d", h=H),
                    mask_sb[:, kt:kt + 1])
                nc.vector.tensor_copy(vaug[:, kt, :, DH:DH + 1],
                                      fbcast(mask_sb[:, kt:kt + 1], H))

            # ---- stage 4: attention per head ----
            scope("attn")
            HB1 = 7  # heads in first normalization batch
            outU = dpool.tile([P, IC, T], F32, tag="xnT")  # reuses xnT's slot
            den0 = bcpool.tile([HB1, T], F32, tag="den0")
            den1 = bcpool.tile([1, T], F32, tag="den1")
            dens = [den0, den1]

            def normalize_batch(b):
                # batched exact reciprocal (rows at partitions 0..n-1)
                n = HB1 if b == 0 else 1
                recb = bcpool.tile([n, T], F32, tag=f"rec{b}")
                nc.vector.reciprocal(recb[0:n, :], dens[b][0:n, :])
                for h in range(0 if b == 0 else HB1, HB1 if b == 0 else H):
                    ic, off = h // 2, (h % 2) * DH
                    r = h if b == 0 else 0
                    # DMA (no partition-start limits) moves row r to partition 0
                    rtmp = chpool.tile([1, T], F32, tag="rtmp")
                    nc.sync.dma_start(out=rtmp[0:1, :], in_=recb[r:r + 1, :])
                    bc = chpool.tile([P, T], F32, tag="bcs")
                    nc.gpsimd.partition_broadcast(bc[0:P, :], rtmp[0:1, :])
                    nc.vector.tensor_tensor(out=outTn[off:off + DH, ic, :],
                                            in0=outU[off:off + DH, ic, :],
                                            in1=bc[off:off + DH, :], op=AOP.mult)

            for h in range(H):
                ic, off = h // 2, (h % 2) * DH
                po = ps.tile([DH + 1, T], F32, tag="mm")
                for grp in range(KT // JG):
                    psim = ps2.tile([P, JG, T], F32, tag="sim")
                    for j2 in range(JG):
                        jt = grp * JG + j2
                        nc.tensor.matmul(psim[:, j2, :],
                                         kTt[off:off + DH, ic, bass.ts(jt, P)],
                                         qT[off:off + DH, ic, :],
                                         start=True, stop=True)
                    et = epool.tile([P, JG, T], BF16, tag="expT")
                    nc.scalar.activation(et, psim, AFT.Exp)
                    for j2 in range(JG):
                        jt = grp * JG + j2
                        nc.tensor.matmul(po[0:DH + 1, :],
                                         vaug[:, jt, h, :],
                                         et[:, j2, :],
                                         start=(jt == 0), stop=(jt == KT - 1))
                nc.vector.tensor_copy(outU[off:off + DH, ic, :], po[0:DH, :])
                dtmp = chpool.tile([1, T], F32, tag="dtmp")
                nc.vector.tensor_copy(dtmp[0:1, :], po[DH:DH + 1, :])
                b = 0 if h < HB1 else 1
                nc.sync.dma_start(out=dens[b][(h if b == 0 else h - HB1):(h if b == 0 else h - HB1) + 1, :],
                                  in_=dtmp[0:1, :])
                if h == HB1 - 1 or h == H - 1:
                    normalize_batch(0 if h == HB1 - 1 else 1)

            # ---- stage 5: final projection + LayerNorm ----
            scope("final")
            for qc in range(TT_):
                pf = ps.tile([P, DIM], F32, tag="mm")
                for ic in range(IC):
                    nc.tensor.matmul(pf, outTn[:, ic, bass.ts(qc, P)],
                                     wo_sb[:, ic, :],
                                     start=(ic == 0), stop=(ic == IC - 1))
                st = spool.tile([P, 6], F32, tag="st")
                mv = spool.tile([P, 2], F32, tag="mv")
                nc.vector.bn_stats(st, pf)
                nc.vector.bn_aggr(mv, st)
                std = spool.tile([P, 1], F32, tag="std")
                nc.scalar.activation(std, mv[:, 1:2], AFT.Sqrt, bias=eps_t[:, 0:1])
                rstd = spool.tile([P, 1], F32, tag="rstd")
                nc.vector.reciprocal(rstd, std)
                yt = ypool.tile([P, DIM], F32, tag="y")
                nc.vector.tensor_scalar(out=yt, in0=pf, scalar1=mv[:, 0:1],
                                        scalar2=rstd, op0=AOP.subtract, op1=AOP.mult)
                nc.vector.tensor_tensor(out=yt, in0=yt, in1=logb, op=AOP.mult)
                nc.vector.tensor_tensor(out=yt, in0=yt, in1=lobb, op=AOP.add)
                nc.sync.dma_start(out=y_d[bass.ts(qc, P), :], in_=yt)
            stack.close()

    nc.compile()
    return nc


def make_in_maps(x, context, mask, ln_g, ln_b, Wq, Wkv, Wo, lno_g, lno_b):
    x = np.asarray(x, np.float32)
    context = np.asarray(context, np.float32)
    maskf = np.asarray(mask).astype(np.float32)
    Wq = np.ascontiguousarray(np.asarray(Wq, np.float32))
    Wkv = np.asarray(Wkv, np.float32)
    Wk = np.ascontiguousarray(Wkv[:, :INNER])
    Wv = np.ascontiguousarray(Wkv[:, INNER:])
    Wo = np.ascontiguousarray(np.asarray(Wo, np.float32))
    ln_g = np.asarray(ln_g, np.float32)
    ln_b = np.asarray(ln_b, np.float32)
    lno_g = np.asarray(lno_g, np.float32)
    lno_b = np.asarray(lno_b, np.float32)

    in_maps = []
    for c in range(NCORES):
        b, q0 = c // (NCORES // B), (c % (NCORES // B)) * T
        in_maps.append({
            "x_s": np.ascontiguousarray(x[b, q0:q0 + T]),
            "ctx": np.ascontiguousarray(context[b]),
            "maskf": np.ascontiguousarray(maskf[b]),
            "Wq": Wq, "Wk": Wk, "Wv": Wv, "Wo": Wo,
            "ln_g": ln_g, "ln_b": ln_b, "lno_g": lno_g, "lno_b": lno_b,
        })
    return in_maps


_NC = None


def _get_nc():
    global _NC
    if _NC is None:
        _NC = build_program()
    return _NC


def kernel(x, context, mask, ln_g, ln_b, Wq, Wkv, Wo, lno_g, lno_b, **run_kwargs):
    nc = _get_nc()
    in_maps = make_in_maps(x, context, mask, ln_g, ln_b, Wq, Wkv, Wo, lno_g, lno_b)
    res = run_bass_kernel_spmd(nc, in_maps, core_ids=list(range(NCORES)), **run_kwargs)
    out = np.empty((B, N, DIM), np.float32)
    for c in range(NCORES):
        b, q0 = c // (NCORES // B), (c % (NCORES // B)) * T
        out[b, q0:q0 + T] = res.results[c]["y"]
    if run_kwargs:
        kernel.last_results = res
    return out
